# revision 7
# baseline (speedup 1.0000x reference)
"""ConvDualAttention Trainium2 kernel v2 (Bass/Tile), 8-core data-parallel.

Contract: kernel(**inputs) takes the FULL unsharded inputs, shards batch b
across the 8 NeuronCores (one batch per core), and returns the full
(8, 128, 4096) float32 output.

Math (per batch b, per head h, D=128, X=4096):
  y_p   = dwconv3(x) + t_p/s_p           (p in q,k,v; BN folded so that
                                          W_eff_p @ y_p == pw_p @ BN(conv))
  k     = W_eff_k @ y_k ; sk = softmax(k over d)
  kat   = SCALE * q^T @ sk               (SCALE folded into W_q)
  gout  = GW @ q + gb ; sig = sigmoid(gout)
  out_h = v @ kat + sig^T * v
  out   = out_w @ merge(out_h) + out_b

Design highlights (vs the first working version, 214us -> ~148us):
  * all-bf16 matmul operands (halves LDWEIGHTS time and weight DMA)
  * y_qT produced by PE transpose of biased y_q tiles interleaved into the
    conv stage; the t'_q bias rides into R so kat = wtq^T @ R directly:
    the 64 sigma matmuls and the qb (x) sigma correction are gone
  * sigmoid(g) computed as 0.5*(1+tanh(g/2)) so the Act engine runs only
    {Exp, Tanh, Identity} from ONE activation table (a sigmoid/exp mix
    reloads the 1.3us act table every switch)
  * one software-pipelined main loop interleaves K production, softmax
    (exp on Act, Z-reduce on DVE, broadcast-normalize on Pool), the
    R accumulation (emitted 3 tiles late to hide the cross-engine
    latency), and the gate path (gout MM, tanh, v MM, gate STT on DVE)
  * gate stored fp8e4m3 (v scaled 1/16, out_w scaled 8x) and consumed by
    fp8 DoubleRow matmuls (head pairs = 256-deep contraction) in the
    final projection; bf16 output DMA
"""
import numpy as np
import ml_dtypes

import concourse.bass as bass
import concourse.tile as tile
from concourse import bacc, mybir
from concourse.bass_utils import run_bass_kernel_spmd

F32 = mybir.dt.float32
BF16 = mybir.dt.bfloat16
FP8 = mybir.dt.float8e4
AF = mybir.ActivationFunctionType
ALU = mybir.AluOpType

B = 8
DIM = 128
HEADS = 8
INNER = DIM * HEADS
X = 4096
EPS = 1e-5
SCALE = DIM ** -0.5
NT = X // 128           # 32 x-tiles of 128
NC4 = X // 1024         # 4 chunks of 1024

_NC = None
TRACE = False
LAST_EXEC_NS = None


def _bf(a):
    return np.ascontiguousarray(np.asarray(a, np.float32).astype(ml_dtypes.bfloat16))


def _prep(inputs):
    """Host-side weight folding. Returns dict of DRAM input arrays."""
    f = lambda k: np.asarray(inputs[k], np.float32)
    wt = {}
    tprime = {}
    dw_cols = []
    for p in ("q", "k", "v"):
        s = f(p + "_g") / np.sqrt(f(p + "_v") + EPS)        # (128,)
        t = f(p + "_b") - f(p + "_m") * s
        tprime[p] = t / s
        w_eff = f(p + "_pw") * s[None, :]                    # (1024, 128)
        wt[p] = np.ascontiguousarray(w_eff.T)                # (128, 1024)
        dw = f(p + "_dw")[:, 0, :]                           # (128, 3)
        for j in range(3):
            dw_cols.append(dw[:, j].astype(np.float32))
    s_gt = f("gt_g") / np.sqrt(f("gt_v") + EPS)
    t_gt = f("gt_b") - f("gt_m") * s_gt
    gw = f("gt_pw") * (f("gt_dw")[:, 0, 0] * s_gt)[None, :]  # (128, 128)
    gb = f("gt_pw") @ t_gt                                   # (128,)
    w_eff_q = wt["q"].T                                      # (1024, 128)
    gqt = np.concatenate(
        [(gw @ w_eff_q[h * 128:(h + 1) * 128, :]).T for h in range(HEADS)], axis=1
    )                                                        # (128 i, 1024 h*o)
    out_w = f("out_w")                                       # (128, 1024)
    outwt = np.concatenate(
        [np.ascontiguousarray(out_w[:, h * 128:(h + 1) * 128].T) for h in range(HEADS)],
        axis=1,
    )                                                        # (128 e, 1024 h*o)
    wvdm = np.concatenate(
        [wt["v"].T[h * 128:(h + 1) * 128, :] for h in range(HEADS)], axis=1
    )                                                        # (128 d, 1024 h*i)
    diag = np.concatenate([np.diag(c).astype(np.float32) for c in dw_cols], axis=1)  # (128, 1152)
    wtq_s = wt["q"] * SCALE                                  # (128 i, 1024 d)
    # sigmoid(g) = 0.5*(1+tanh(g/2)): tanh shares the Act table with exp.
    # The 0.5 is folded into wtv (v is only used by the gate path) along
    # with a 1/8 fp8-range scale undone by outw8 = 8*out_w; tanh bias is gb/2.
    biasp = np.stack(
        [tprime["q"], tprime["k"], tprime["v"], 0.5 * gb, f("out_b")], axis=1
    )                                                        # (128, 5)
    return {
        "wtk": _bf(wt["k"]),
        "wtv": _bf(wt["v"] / 16.0),
        "gqt": _bf(gqt),
        "outwt": _bf(outwt),
        "wvdm": _bf(wvdm),
        "diag": _bf(diag),
        "biasp": np.ascontiguousarray(biasp.astype(np.float32)),
        "wtqr": _bf(wtq_s),
        "eye": _bf(np.eye(128, dtype=np.float32)),
        "outw8": np.ascontiguousarray(
            (8.0 * outwt).astype(ml_dtypes.float8_e4m3)
        ),
    }


def _build():
    nc = bacc.Bacc("TRN2", target_bir_lowering=False, debug=False, num_devices=B)
    xb_d = nc.dram_tensor("xb", [128, X + 2], BF16, kind="ExternalInput").ap()
    wtk_d = nc.dram_tensor("wtk", [128, INNER], BF16, kind="ExternalInput").ap()
    wtv_d = nc.dram_tensor("wtv", [128, INNER], BF16, kind="ExternalInput").ap()
    gqt_d = nc.dram_tensor("gqt", [128, INNER], BF16, kind="ExternalInput").ap()
    outwt_d = nc.dram_tensor("outwt", [128, INNER], BF16, kind="ExternalInput").ap()
    wvdm_d = nc.dram_tensor("wvdm", [128, INNER], BF16, kind="ExternalInput").ap()
    diag_d = nc.dram_tensor("diag", [128, 9 * 128], BF16, kind="ExternalInput").ap()
    biasp_d = nc.dram_tensor("biasp", [128, 5], F32, kind="ExternalInput").ap()
    wtqr_d = nc.dram_tensor("wtqr", [128, INNER], BF16, kind="ExternalInput").ap()
    eye_d = nc.dram_tensor("eye", [128, 128], BF16, kind="ExternalInput").ap()
    outw8_d = nc.dram_tensor("outw8", [128, INNER], FP8, kind="ExternalInput").ap()
    out_d = nc.dram_tensor("out", [128, X], BF16, kind="ExternalOutput").ap()

    with tile.TileContext(nc) as tc:
        with (
            tc.tile_pool(name="const", bufs=1) as cp,
        ):
            wtk = cp.tile([128, INNER], BF16)
            wtv = cp.tile([128, INNER], BF16)
            gqt = cp.tile([128, INNER], BF16)
            outwt = cp.tile([128, INNER], BF16)
            wvdm = cp.tile([128, INNER], BF16)
            wtqr = cp.tile([128, INNER], BF16)
            diag = cp.tile([128, 9 * 128], BF16)
            biasp = cp.tile([128, 5], F32)
            eye = cp.tile([128, 128], BF16)
            outw8 = cp.tile([128, INNER], FP8)
            xb = cp.tile([128, X + 2], BF16)
            yq = cp.tile([128, X], BF16, tag="yq")
            yk = cp.tile([128, X], BF16, tag="yk")
            yv = cp.tile([128, X], BF16, tag="yv")
            yqt = cp.tile([128, X], BF16, tag="yqt")
            zt = cp.tile([128, NT * HEADS], F32, tag="zt")
            zif = cp.tile([128, NT * HEADS], F32, tag="zif")
            gate = cp.tile([128, HEADS * X], FP8, tag="gate")
            r_sb = cp.tile([128, INNER], BF16, tag="rsb")
            ones8 = cp.tile([128, 8], F32, tag="ones8")
            w3t_sb = cp.tile([128, 128], BF16, tag="w3t")

            # xb in chunks so conv c=0 can start before the full input lands;
            # spread input DMAs over idle engine queues to avoid SP serialization
            nc.sync.dma_start(out=diag, in_=diag_d)
            nc.scalar.dma_start(out=biasp, in_=biasp_d)
            nc.sync.dma_start(out=xb, in_=xb_d)
            nc.scalar.dma_start(out=eye, in_=eye_d)
            for eng, sb_t, dr in ((nc.scalar, wtk, wtk_d), (nc.gpsimd, wtqr, wtqr_d),
                                  (nc.scalar, wvdm, wvdm_d), (nc.gpsimd, outwt, outwt_d),
                                  (nc.scalar, gqt, gqt_d), (nc.scalar, wtv, wtv_d),
                                  (nc.gpsimd, outw8, outw8_d)):
                eng.dma_start(out=sb_t, in_=dr)

            nc.gpsimd.memset(ones8, 1.0)

            ys = {"q": yq, "k": yk, "v": yv}

            # ---- conv + transpose stage: depthwise conv via 3 shifted diagonal
            # matmuls; yq tiles are PE-transposed right after each q chunk so
            # yqt evacs ride the otherwise-idle DVE ----
            with (
                tc.tile_pool(name="cvp", bufs=3, space="PSUM") as cvp,
                tc.tile_pool(name="tpp", bufs=2, space="PSUM") as tpp,
            ):
                def emit_transpose_group(g):
                    tp = tpp.tile([128, 512], BF16, tag="tp", name="tp")
                    for i in range(4):
                        t = g * 4 + i
                        nc.tensor.transpose(
                            tp[:, i * 128:(i + 1) * 128],
                            yq[:, t * 128:(t + 1) * 128], eye,
                        )
                    nc.vector.tensor_copy(yqt[:, g * 512:(g + 1) * 512], tp)

                for pi, p in enumerate(("q", "k", "v")):
                    for c in range(NC4):
                        pt = cvp.tile([128, 1024], F32, tag="cv")
                        for j in range(3):
                            dsl = diag[:, (pi * 3 + j) * 128:(pi * 3 + j + 1) * 128]
                            for u in range(2):
                                x0 = c * 1024 + u * 512 + j
                                nc.tensor.matmul(
                                    pt[:, u * 512:(u + 1) * 512], dsl,
                                    xb[:, x0:x0 + 512],
                                    start=(j == 0), stop=(j == 2),
                                )
                        nc.scalar.activation(
                            ys[p][:, c * 1024:(c + 1) * 1024], pt,
                            AF.Identity, bias=biasp[:, pi:pi + 1],
                        )
                        if p == "q" and c >= 1:
                            emit_transpose_group(2 * (c - 1))
                            emit_transpose_group(2 * (c - 1) + 1)
                if True:
                    emit_transpose_group(6)
                    emit_transpose_group(7)

            # ---- main loop: K, softmax, R accumulation + gate production ----
            with (
                tc.tile_pool(name="kqp", bufs=1, space="PSUM") as kqp,
                tc.tile_pool(name="rp", bufs=1, space="PSUM") as rp,
                tc.tile_pool(name="gop", bufs=1, space="PSUM") as gop,
                tc.tile_pool(name="vpp", bufs=1, space="PSUM") as vpp,
                tc.tile_pool(name="skp", bufs=6) as skp,
                tc.tile_pool(name="sgp", bufs=3) as sgp,
            ):
                r_ps = rp.tile([128, INNER], F32, tag="r")
                sks = [None] * NT       # (pair_tile, half) per t
                vps = [None] * NT
                sgs = [None] * NT

                def emit_R(t):
                    for u in range(2):
                        nc.tensor.matmul(
                            r_ps[:, u * 512:(u + 1) * 512],
                            yqt[:, t * 128:(t + 1) * 128],
                            sks[t][:, u * 512:(u + 1) * 512],
                            start=(t == 0), stop=(t == NT - 1),
                            skip_group_check=True,
                        )

                def emit_gate(t):
                    # gate = (1 + tanh(gout/2 + gb/2)) * (v/2) == sigmoid(gout+gb) * v
                    gsl = gate[:, t * 1024:(t + 1) * 1024]
                    nc.vector.scalar_tensor_tensor(
                        gsl, sgs[t], 1.0, vps[t], ALU.add, ALU.mult
                    )

                for t in range(NT):
                    kq = kqp.tile([128, INNER], F32, tag="kq")
                    for u in range(2):
                        nc.tensor.matmul(
                            kq[:, u * 512:(u + 1) * 512],
                            yk[:, t * 128:(t + 1) * 128],
                            wtk[:, u * 512:(u + 1) * 512],
                            start=True, stop=True, skip_group_check=True,
                        )
                    if t >= 3:
                        emit_R(t - 3)
                    sk = skp.tile([128, INNER], BF16, tag="sk")
                    sks[t] = sk
                    nc.scalar.activation(sk, kq, AF.Exp)
                    skv = sk.rearrange("p (h d) -> p h d", h=HEADS)
                    nc.vector.tensor_reduce(
                        zt[:, t * 8:(t + 1) * 8], skv,
                        mybir.AxisListType.X, ALU.add,
                    )
                    nc.vector.reciprocal(
                        zif[:, t * 8:(t + 1) * 8], zt[:, t * 8:(t + 1) * 8]
                    )
                    zin = zif[:, t * 8:(t + 1) * 8].unsqueeze(2).broadcast_to(
                        [128, 8, 128]
                    )
                    nc.gpsimd.tensor_tensor(skv, skv, zin, ALU.mult)
                    # gate production for (h, c2) = (t // 4, t % 4)
                    h, c2 = t // 4, t % 4
                    go = gop.tile([128, 1024], F32, tag="go")
                    for u in range(2):
                        x0 = c2 * 1024 + u * 512
                        nc.tensor.matmul(
                            go[:, u * 512:(u + 1) * 512],
                            gqt[:, h * 128:(h + 1) * 128], yq[:, x0:x0 + 512],
                            start=True, stop=True, skip_group_check=True,
                        )
                    sig = sgp.tile([128, 1024], BF16, tag="sig")
                    sgs[t] = sig
                    nc.scalar.activation(sig, go, AF.Tanh, bias=biasp[:, 3:4],
                                         scale=0.5)
                    vp = vpp.tile([128, 1024], F32, tag="vp")
                    vps[t] = vp
                    for u in range(2):
                        x0 = c2 * 1024 + u * 512
                        nc.tensor.matmul(
                            vp[:, u * 512:(u + 1) * 512],
                            wtv[:, h * 128:(h + 1) * 128], yv[:, x0:x0 + 512],
                            start=True, stop=True, skip_group_check=True,
                        )
                    if t >= 1:
                        emit_gate(t - 1)
                emit_R(NT - 3)
                emit_R(NT - 2)
                emit_gate(NT - 1)
                emit_R(NT - 1)
                nc.vector.tensor_copy(r_sb, r_ps)

            # ---- kat -> M2 -> W3T per head ----
            with (
                tc.tile_pool(name="katp", bufs=2, space="PSUM") as katp,
                tc.tile_pool(name="m2p", bufs=2, space="PSUM") as m2p,
                tc.tile_pool(name="w3p", bufs=1, space="PSUM") as w3p,
                tc.tile_pool(name="small", bufs=2) as sp,
            ):
                w3t_ps = w3p.tile([128, 128], F32)
                kat_ps_l = [None] * HEADS
                kat_sb_l = [None] * HEADS
                m2_ps_l = [None] * HEADS
                m2_sb_l = [None] * HEADS

                def emit_kat(h):
                    hsl = slice(h * 128, (h + 1) * 128)
                    kat_ps_l[h] = katp.tile([128, 128], F32, tag="katp", name="kat_ps")
                    nc.tensor.matmul(
                        kat_ps_l[h], wtqr[:, hsl], r_sb[:, hsl],
                        start=True, stop=True, skip_group_check=True,
                    )
                    kat_sb_l[h] = sp.tile([128, 128], BF16, tag="katsb", name="kat_sb")
                    nc.vector.tensor_copy(kat_sb_l[h], kat_ps_l[h])

                def emit_m2(h):
                    hsl = slice(h * 128, (h + 1) * 128)
                    m2_ps_l[h] = m2p.tile([128, 128], F32, tag="m2", name="m2_ps")
                    nc.tensor.matmul(
                        m2_ps_l[h], kat_sb_l[h], wvdm[:, hsl],
                        start=True, stop=True, skip_group_check=True,
                    )
                    m2_sb_l[h] = sp.tile([128, 128], BF16, tag="m2sb", name="m2_sb")
                    nc.vector.tensor_copy(m2_sb_l[h], m2_ps_l[h])

                def emit_w3(h):
                    nc.tensor.matmul(
                        w3t_ps, m2_sb_l[h], outwt[:, h * 128:(h + 1) * 128],
                        start=(h == 0), stop=(h == HEADS - 1),
                        skip_group_check=True,
                    )

                for h in range(HEADS):
                    emit_kat(h)
                    if h >= 1:
                        emit_m2(h - 1)
                    if h >= 2:
                        emit_w3(h - 2)
                emit_m2(HEADS - 1)
                emit_w3(HEADS - 2)
                emit_w3(HEADS - 1)
                nc.vector.tensor_copy(w3t_sb, w3t_ps)

            # ---- final projection per 1024-chunk ----
            with (
                tc.tile_pool(name="finps", bufs=2, space="PSUM") as finps,
                tc.tile_pool(name="bpool", bufs=2) as bp,
            ):
                gv = gate.rearrange("p (hp r c x) -> p hp r c x",
                                    hp=HEADS // 2, r=2, c=NC4)
                for c2 in range(NC4):
                    fin_ps = finps.tile([128, 1024], F32, tag="fin")
                    # gate-part first: it does not depend on w3t, so it
                    # overlaps the kat phase
                    for hp in range(HEADS // 2):
                        lhs8 = outw8[:, hp * 256:(hp + 1) * 256].rearrange(
                            "p (r m) -> p r m", r=2
                        )
                        for u in range(2):
                            rhs8 = gv[:, hp:hp + 1, :, c2:c2 + 1,
                                      u * 512:(u + 1) * 512].squeeze()
                            nc.tensor.matmul(
                                fin_ps[:, u * 512:(u + 1) * 512],
                                lhs8, rhs8,
                                start=(hp == 0), stop=False,
                                perf_mode=mybir.MatmulPerfMode.DoubleRow,
                                skip_group_check=True,
                            )
                    for u in range(2):
                        x0 = c2 * 1024 + u * 512
                        nc.tensor.matmul(
                            fin_ps[:, u * 512:(u + 1) * 512], w3t_sb,
                            yv[:, x0:x0 + 512],
                            start=False, stop=True, skip_group_check=True,
                        )
                    csl = slice(c2 * 1024, (c2 + 1) * 1024)
                    fin_sb = bp.tile([128, 1024], BF16, tag="finsb")
                    if c2 % 2 == 0:
                        nc.scalar.activation(
                            fin_sb, fin_ps, AF.Identity, bias=biasp[:, 4:5]
                        )
                    else:
                        nc.vector.tensor_scalar_add(fin_sb, fin_ps, biasp[:, 4:5])
                    for u in range(2):
                        osl = slice(c2 * 1024 + u * 512, c2 * 1024 + (u + 1) * 512)
                        nc.sync.dma_start(
                            out=out_d[:, osl], in_=fin_sb[:, u * 512:(u + 1) * 512]
                        )

    nc.compile()
    return nc


def kernel(**inputs):
    global _NC, LAST_EXEC_NS
    host = _prep(inputs)
    if _NC is None:
        _NC = _build()
    x = np.asarray(inputs["x"], np.float32)
    in_maps = []
    for b in range(B):
        xp = np.pad(x[b], ((0, 0), (1, 1)))
        m = {"xb": _bf(xp)}
        m.update(host)
        in_maps.append(m)
    res = run_bass_kernel_spmd(
        _NC, in_maps, core_ids=list(range(B)), trace=TRACE
    )
    LAST_EXEC_NS = res.exec_time_ns
    return np.stack([r["out"] for r in res.results]).astype(np.float32)
